# revision 61
# baseline (speedup 1.0000x reference)
"""Distributed attention layer kernel for 8 TRN2 NeuronCores.

Reference computation (f32):
    Q = q @ W_q; K = k @ W_k; V = v @ W_v
    out = softmax((Q @ K^T)/sqrt(d_k)) @ V

Sharding: rows of q/k/v are split 8 ways (sequence parallel). Each core
projects its own shards, the K^T/V projections are all-gathered (bf16),
and each core computes its 512-row slice of the attention output.

Precision: the Q/K/V projections run as single-pass float32r matmuls
(~12-bit mantissa, 1 cycle/row for 512-wide outputs — measured 227 ns
per [128x128]x[128x512] matmul vs 215 ns fp16) with f32 PSUM, so Q/K
land correctly-rounded fp16 for the score path at a third of the
compensated-split cost. Q@K^T is a single fp16 matmul (f32 PSUM),
~4e-3 end-to-end vs the 2e-2 gate. The V path is fp16. Softmax is f32
(ACT exp with per-row max bias, fused row-sum).
"""

import os
import sys

for _p in ("/opt/pypackages", "/opt/trn_rl_repo"):
    if _p not in sys.path:
        sys.path.insert(0, _p)

import numpy as np

N_Q, N_KV, DIM = 4096, 4096, 1024  # D_K = D_V = DIM (square weights)
CORES = 8

P = 128


def build_attention(nq=N_Q, dim=DIM, cores=CORES):
    """Build the per-core Bass graph (SPMD; identical on all cores)."""
    import concourse.bass as bass
    import concourse.mybir as mybir
    from concourse import bacc
    from concourse.masks import make_identity
    from concourse.tile import TileContext

    dt = mybir.dt
    f32, bf16 = dt.float32, dt.float16  # "bf16" vars are fp16 now
    f32r = dt.float32r

    sh = nq // cores          # rows per core (512)
    n_ct = dim // P           # contraction tiles for projections (8)
    n_dt = dim // P           # d tiles (8)
    n_it = sh // P            # query-row tiles per core (4)
    n_jjt = sh // P           # kv-row tiles per core (4)
    n_eh = dim // 512         # 512-wide output column halves (2)
    EH = 512 if dim >= 512 else dim
    n_eh = max(1, dim // EH)
    n_jt = nq // P            # total kv j tiles (32)
    JG = 4                    # j-tiles per PV V-chunk
    n_jg = n_jt // JG         # V chunk count (8)
    IT_GROUP = 2              # i-tiles per PV psum group
    scale = 1.0 / float(np.sqrt(dim))

    nc = bacc.Bacc(num_devices=cores)

    # --- external I/O (per core: row shards of q/k/v, full weights) ---
    q_ext = nc.declare_dram_parameter("q", [sh, dim], f32, isOutput=False)
    k_ext = nc.declare_dram_parameter("k", [sh, dim], f32, isOutput=False)
    v_ext = nc.declare_dram_parameter("v", [sh, dim], f32, isOutput=False)
    wq_ext = nc.declare_dram_parameter("W_q", [dim, dim], f32, isOutput=False)
    wk_ext = nc.declare_dram_parameter("W_k", [dim, dim], f32, isOutput=False)
    wv_ext = nc.declare_dram_parameter("W_v", [dim, dim], f32, isOutput=False)
    out_ext = nc.declare_dram_parameter("out", [sh, dim], f32, isOutput=True)

    # --- internal DRAM for collectives ---
    # bounce/gather layouts are partition-major so every DMA touches
    # 4-8 KB contiguous runs per partition (few, large descriptors).
    # Exactly TWO collectives: ticks of first and last collectives post
    # promptly; middle ticks were observed posting ~30us late.
    n_dtt = dim // P   # d-tiles (8)
    n_vjt = sh // P    # j-tiles per core (4)
    bounce_k = nc.dram_tensor("bounce_k", [P * n_dtt, sh], bf16)
    bounce_v = nc.dram_tensor("bounce_v", [P * n_vjt, dim], bf16)
    gath_k = nc.dram_tensor("gath_k", [cores * P * n_dtt, sh], bf16, addr_space="Shared")
    gath_v = nc.dram_tensor("gath_v", [cores * P * n_vjt, dim], bf16, addr_space="Shared")

    rg = [list(range(cores))]

    with TileContext(nc) as tc:
        with (
            tc.tile_pool(name="const", bufs=1) as constp,
            tc.tile_pool(name="qt", bufs=1) as qtp,
            tc.tile_pool(name="stats", bufs=1) as statp,
        ):
            ident_f = constp.tile([P, P], f32, tag="idf", name="idf")
            make_identity(nc, ident_f)
            ident_b = constp.tile([P, P], bf16, tag="idb", name="idb")
            make_identity(nc, ident_b)

            qthi = qtp.tile([P, n_dt, sh], bf16, tag="qthi", name="qthi")

            def load_w_f32r(w_ext, wpool, wstage):
                """Load a [dim, dim] f32 weight into SBUF as float32r laid
                out [c_in=128, ct, d] (scalar copy performs the rounding)."""
                wr = wpool.tile([P, n_ct, dim], f32r, tag="wr", name="wr")
                wsrc = w_ext.rearrange("(ct p) d -> p ct d", p=P)
                for ct in range(n_ct):
                    stg = wstage.tile([P, dim], f32, tag="wstg", name="wstg")
                    nc.sync.dma_start(stg[:], wsrc[:, ct])
                    nc.scalar.copy(wr[:, ct], stg[:])
                return wr

            def load_transpose_f32r(x_ext, tpool, iost, tpsum):
                """Load a [sh, dim] f32 input, transpose on PE, round to
                float32r in SBUF laid out [c_in=128, ct, row]."""
                xt = tpool.tile([P, n_ct, sh], f32r, tag="xt", name="xt")
                xsrc = x_ext.rearrange("(it p) c -> p it c", p=P)
                for it in range(sh // P):
                    stg = iost.tile([P, dim], f32, tag="iostg", name="iostg")
                    nc.sync.dma_start(stg[:], xsrc[:, it])
                    for ct in range(n_ct):
                        ps = tpsum.tile([P, P], f32, tag="tps", name="tps")
                        nc.tensor.transpose(ps[:], stg[:, ct * P:(ct + 1) * P], ident_f)
                        nc.vector.tensor_copy(xt[:, ct, it * P:(it + 1) * P], ps[:])
                return xt

            with (
                tc.tile_pool(name="wstage", bufs=1) as wstage,
                tc.tile_pool(name="w", bufs=2) as wpool,
                tc.tile_pool(name="iost", bufs=4) as iost,
                tc.tile_pool(name="tin", bufs=2) as tpool,
                tc.tile_pool(name="kvout", bufs=2) as kvout,
                tc.tile_pool(name="tpsum", bufs=2, space="PSUM") as tpsum,
                tc.tile_pool(name="ppsum", bufs=1, space="PSUM") as ppsum,
                tc.tile_pool(name="kproj", bufs=4, space="PSUM") as kprojp,
            ):
                # ---- K path: project K^T shard (f32r matmul, fp16
                # correctly-rounded result), bounce out, AG ----
                # The k transposes interleave with the projection ct-by-ct
                # (two dtt-half passes, 4 open PSUM groups each) so the PE
                # pipeline never serializes transpose-then-project and the
                # K all-gather triggers as early as possible.
                ksrc4 = k_ext.rearrange("(it p) c -> p it c", p=P)
                kstg = []
                for it4 in range(sh // P):
                    stg = iost.tile([P, dim], f32, tag="iostg", name="iostg")
                    nc.sync.dma_start(stg[:], ksrc4[:, it4])
                    kstg.append(stg)
                wkr = load_w_f32r(wk_ext, wpool, wstage)
                ktr = tpool.tile([P, n_ct, sh], f32r, tag="xt", name="ktr")

                kt_hi_loc = kvout.tile([P, n_dt, sh], bf16, tag="kthi_loc", name="kthi_loc")
                for half in range(2):
                    dtts = range(half * (n_dt // 2), (half + 1) * (n_dt // 2))
                    pss = {
                        dtt: kprojp.tile([P, sh], f32, tag="kpps", name="kpps")
                        for dtt in dtts
                    }
                    for ct in range(n_ct):
                        if half == 0:
                            for it4 in range(sh // P):
                                tps = tpsum.tile([P, P], f32, tag="tps", name="tps")
                                nc.tensor.transpose(
                                    tps[:], kstg[it4][:, ct * P:(ct + 1) * P],
                                    ident_f,
                                )
                                nc.vector.tensor_copy(
                                    ktr[:, ct, it4 * P:(it4 + 1) * P], tps[:]
                                )
                        for dtt in dtts:
                            dsl = slice(dtt * P, (dtt + 1) * P)
                            nc.tensor.matmul(
                                pss[dtt][:], wkr[:, ct, dsl], ktr[:, ct],
                                start=(ct == 0), stop=(ct == n_ct - 1),
                            )
                    for dtt in dtts:
                        nc.scalar.copy(kt_hi_loc[:, dtt], pss[dtt][:])

                bk = bounce_k.rearrange("(p dtt) jj -> p dtt jj", p=P)
                nc.sync.dma_start(bk[:], kt_hi_loc[:])
                nc.gpsimd.collective_compute(
                    "AllGather", mybir.AluOpType.bypass, replica_groups=rg,
                    ins=[bounce_k.ap().opt()], outs=[gath_k.ap().opt()],
                )

                # ---- V path: project V shard (f32r), bounce, AG right
                # behind the K gathers so it lands before PV needs it ----
                wvr = load_w_f32r(wv_ext, wpool, wstage)
                vtr = load_transpose_f32r(v_ext, tpool, iost, tpsum)

                v_loc = kvout.tile([P, n_jjt, dim], bf16, tag="v_loc", name="v_loc")
                for jjt in range(n_jjt):
                    jsl = slice(jjt * P, (jjt + 1) * P)
                    for eh in range(n_eh):
                        ps = ppsum.tile([P, EH], f32, tag="ppsv", name="ppsv")
                        esl = slice(eh * EH, (eh + 1) * EH)
                        for ct in range(n_ct):
                            nc.tensor.matmul(
                                ps[:], vtr[:, ct, jsl], wvr[:, ct, esl],
                                start=(ct == 0), stop=(ct == n_ct - 1),
                            )
                        nc.scalar.copy(v_loc[:, jjt, esl], ps[:])

                bv = bounce_v.rearrange("(p jjt) e -> p jjt e", p=P)
                nc.sync.dma_start(bv[:], v_loc[:])
                # (the V all-gather itself is issued after the score chunk
                # loads: any DRAM read issued after a collective waits for
                # its tick, so issuing AG_v here would stall the khi loads)

                # ---- Q path (local only): project Q^T (f32r) ----
                wqr = load_w_f32r(wq_ext, wpool, wstage)
                qtr = load_transpose_f32r(q_ext, tpool, iost, tpsum)

                for dtt in range(n_dt):
                    ps = ppsum.tile([P, sh], f32, tag="pps", name="pps")
                    dsl = slice(dtt * P, (dtt + 1) * P)
                    for ct in range(n_ct):
                        nc.tensor.matmul(
                            ps[:], wqr[:, ct, dsl], qtr[:, ct],
                            start=(ct == 0), stop=(ct == n_ct - 1),
                        )
                    nc.scalar.copy(qthi[:, dtt], ps[:])

            # ================= attention phase =================
            m_t = [
                [
                    statp.tile([P, 1], f32, tag=f"m{it}_{pp}", name=f"m{it}_{pp}")
                    for pp in range(2)
                ]
                for it in range(n_it)
            ]
            bias_t = [statp.tile([P, 1], f32, tag=f"b{it}", name=f"b{it}") for it in range(n_it)]
            ell_t = [statp.tile([P, 1], f32, tag=f"l{it}", name=f"l{it}") for it in range(n_it)]
            rl_t = [statp.tile([P, 1], f32, tag=f"r{it}", name=f"r{it}") for it in range(n_it)]

            gk = gath_k.rearrange("(r p dtt) jj -> r p dtt jj", r=cores, p=P)
            gv = gath_v.rearrange("(r p jj) e -> r p jj e", r=cores, p=P)

            with (
                tc.tile_pool(name="prow", bufs=n_it) as prow,
                tc.tile_pool(name="ptp", bufs=1) as ptp,
                tc.tile_pool(name="vchunk", bufs=3) as vchunk,
                tc.tile_pool(name="opool", bufs=2) as opool,
            ):
                _schunk_cm = tc.tile_pool(name="schunk", bufs=1)
                schunk = _schunk_cm.__enter__()
                _srow_cm = tc.tile_pool(name="srow", bufs=2)
                srow = _srow_cm.__enter__()

                # ---- scores: the whole gathered K^T comes in as just TWO
                # big DMAs (fewest instructions => fewest chances for the
                # scheduler's completion-count thresholds to couple them to
                # the V collective); V all-gather issued after them ----
                _spsum_cm = tc.tile_pool(name="spsum", bufs=6, space="PSUM")
                spsum = _spsum_cm.__enter__()
                _ptpsum_cm = tc.tile_pool(name="ptpsum", bufs=2, space="PSUM")
                ptpsum = _ptpsum_cm.__enter__()
                gkb = gath_k.rearrange("(r p dtt) jj -> p r dtt jj", r=cores, p=P)
                # four quarter loads alternating HWDGE queues: the first
                # 2 MB lands ~10us after the K gather, and quarters arrive
                # in exactly the order the it-outer score loop consumes them
                qc = cores // 4
                khi_q = []
                for qi in range(4):
                    kq = schunk.tile(
                        [P, qc, n_dt, sh], bf16, tag=f"kq{qi}", name=f"kq{qi}"
                    )
                    eng = nc.sync if qi % 2 == 0 else nc.scalar
                    eng.dma_start(kq[:], gkb[:, qi * qc:(qi + 1) * qc])
                    khi_q.append(kq)
                nc.gpsimd.collective_compute(
                    "AllGather", mybir.AluOpType.bypass, replica_groups=rg,
                    ins=[bounce_v.ap().opt()], outs=[gath_v.ap().opt()],
                )

                # it-outer: S rows complete one i-tile at a time, so exp and
                # the P^T transposes for tile i overlap the scores of i+1
                p_sb = []
                pts = {}

                def emit_transposes(it):
                    pt = ptp.tile([P, n_jt, P], bf16, tag=f"pt{it}", name=f"pt{it}")
                    for jt in range(n_jt):
                        tps = ptpsum.tile([P, P], bf16, tag="ptps", name="ptps")
                        nc.tensor.transpose(
                            tps[:], p_sb[it][:, jt * P:(jt + 1) * P], ident_b
                        )
                        nc.vector.tensor_copy(pt[:, jt], tps[:])
                    pts[it] = pt

                for it in range(n_it):
                    isl = slice(it * P, (it + 1) * P)
                    s_it = srow.tile([P, nq], f32, tag="s", name="s")
                    for rr in range(cores):
                        ksrc = khi_q[rr // qc]
                        kidx = rr % qc
                        ps = spsum.tile([P, sh], f32, tag="sps", name="sps")
                        for dtt in range(n_dt):
                            nc.tensor.matmul(
                                ps[:], qthi[:, dtt, isl], ksrc[:, kidx, dtt],
                                start=(dtt == 0), stop=(dtt == n_dt - 1),
                            )
                        if rr == 0:
                            nc.vector.reduce_max(
                                m_t[it][0][:], ps[:], axis=mybir.AxisListType.X
                            )
                        else:
                            nc.vector.reduce_max(
                                m_t[it][1][:], ps[:], axis=mybir.AxisListType.X
                            )
                            nc.vector.tensor_max(
                                m_t[it][0][:], m_t[it][0][:], m_t[it][1][:]
                            )
                        nc.scalar.copy(
                            s_it[:, rr * sh:(rr + 1) * sh], ps[:]
                        )
                    p_it = prow.tile([P, nq], bf16, tag="p", name="p")
                    p_sb.append(p_it)
                    nc.vector.tensor_scalar_mul(
                        bias_t[it][:], m_t[it][0][:], -scale
                    )
                    nc.scalar.activation(
                        p_it[:], s_it[:],
                        mybir.ActivationFunctionType.Exp,
                        bias=bias_t[it][:], scale=scale,
                        accum_out=ell_t[it][:],
                    )
                    nc.vector.reciprocal(rl_t[it][:], ell_t[it][:])
                    if it >= 1:
                        emit_transposes(it - 1)
                emit_transposes(n_it - 1)

                _ptpsum_cm.__exit__(None, None, None)
                _spsum_cm.__exit__(None, None, None)
                _srow_cm.__exit__(None, None, None)
                _schunk_cm.__exit__(None, None, None)
                # ---- O = (P @ V) / ell, single V pass, 8 psum banks ----

                _pvpsum_cm = tc.tile_pool(name="pvpsum", bufs=n_it * n_eh, space="PSUM")
                pvpsum = _pvpsum_cm.__enter__()
                pso = {
                    (it, eh): pvpsum.tile([P, EH], f32, tag="pvps", name="pvps")
                    for it in range(n_it) for eh in range(n_eh)
                }
                # prefetch ALL V chunks across both HWDGE queues so the PV
                # matmul stream is purely PE-bound once the gather lands
                vcs = []
                for r in range(cores):
                    vc = vchunk.tile([P, n_jjt, dim], bf16, tag="vc", name="vc")
                    eng = nc.scalar if r % 2 == 0 else nc.sync
                    eng.dma_start(vc[:], gv[r])
                    vcs.append(vc)
                for r in range(cores):
                    for jj in range(n_jjt):
                        jt = r * n_jjt + jj
                        for it in range(n_it):
                            for eh in range(n_eh):
                                esl = slice(eh * EH, (eh + 1) * EH)
                                nc.tensor.matmul(
                                    pso[(it, eh)][:],
                                    pts[it][:, jt],
                                    vcs[r][:, jj, esl],
                                    start=(r == 0 and jj == 0),
                                    stop=(r == cores - 1 and jj == n_jjt - 1),
                                )
                for it in range(n_it):
                    o_sb = opool.tile([P, dim], f32, tag="o", name="o")
                    for eh in range(n_eh):
                        esl = slice(eh * EH, (eh + 1) * EH)
                        nc.vector.tensor_scalar_mul(
                            o_sb[:, esl], pso[(it, eh)][:], rl_t[it][:]
                        )
                    nc.sync.dma_start(out_ext[it * P:(it + 1) * P, :], o_sb[:])
                _pvpsum_cm.__exit__(None, None, None)

    return nc


_CACHE = {}
RUN_KW = {}


def _get_nc():
    if "nc" not in _CACHE:
        _CACHE["nc"] = build_attention()
    return _CACHE["nc"]


def kernel(**inputs):
    from concourse.bass_utils import run_bass_kernel_spmd

    q = np.ascontiguousarray(np.asarray(inputs["q"], dtype=np.float32))
    k = np.ascontiguousarray(np.asarray(inputs["k"], dtype=np.float32))
    v = np.ascontiguousarray(np.asarray(inputs["v"], dtype=np.float32))
    W_q = np.ascontiguousarray(np.asarray(inputs["W_q"], dtype=np.float32))
    W_k = np.ascontiguousarray(np.asarray(inputs["W_k"], dtype=np.float32))
    W_v = np.ascontiguousarray(np.asarray(inputs["W_v"], dtype=np.float32))

    sh = N_Q // CORES
    in_maps = []
    for r in range(CORES):
        sl = slice(r * sh, (r + 1) * sh)
        in_maps.append({
            "q": q[sl], "k": k[sl], "v": v[sl],
            "W_q": W_q, "W_k": W_k, "W_v": W_v,
        })

    nc = _get_nc()
    if not nc.is_finalized():
        nc.finalize()
    res = run_bass_kernel_spmd(nc, in_maps, core_ids=list(range(CORES)), **RUN_KW)
    _CACHE["last_result"] = res
    out = np.concatenate([res.results[r]["out"] for r in range(CORES)], axis=0)
    return out


if __name__ == "__main__":
    import reference

    inputs = {kk: np.asarray(vv) for kk, vv in reference.setup_inputs().items()}
    out = kernel(**inputs)
    print("out shape:", out.shape, out.dtype)



# revision 62
# speedup vs baseline: 1.0942x; 1.0942x over previous
"""Distributed attention layer kernel for 8 TRN2 NeuronCores.

Reference computation (f32):
    Q = q @ W_q; K = k @ W_k; V = v @ W_v
    out = softmax((Q @ K^T)/sqrt(d_k)) @ V

Sharding: rows of q/k/v are split 8 ways (sequence parallel). Each core
projects its own shards, the K^T/V projections are all-gathered (bf16),
and each core computes its 512-row slice of the attention output.

Precision: the Q/K/V projections run as single-pass float32r matmuls
(~12-bit mantissa, 1 cycle/row for 512-wide outputs — measured 227 ns
per [128x128]x[128x512] matmul vs 215 ns fp16) with f32 PSUM, so Q/K
land correctly-rounded fp16 for the score path at a third of the
compensated-split cost. Q@K^T is a single fp16 matmul (f32 PSUM),
~4e-3 end-to-end vs the 2e-2 gate. The V path is fp16. Softmax is f32
(ACT exp with per-row max bias, fused row-sum).
"""

import os
import sys

for _p in ("/opt/pypackages", "/opt/trn_rl_repo"):
    if _p not in sys.path:
        sys.path.insert(0, _p)

import numpy as np

N_Q, N_KV, DIM = 4096, 4096, 1024  # D_K = D_V = DIM (square weights)
CORES = 8

P = 128


def build_attention(nq=N_Q, dim=DIM, cores=CORES):
    """Build the per-core Bass graph (SPMD; identical on all cores)."""
    import concourse.bass as bass
    import concourse.mybir as mybir
    from concourse import bacc
    from concourse.masks import make_identity
    from concourse.tile import TileContext

    dt = mybir.dt
    f32, bf16 = dt.float32, dt.float16  # "bf16" vars are fp16 now
    f32r = dt.float32r

    sh = nq // cores          # rows per core (512)
    n_ct = dim // P           # contraction tiles for projections (8)
    n_dt = dim // P           # d tiles (8)
    n_it = sh // P            # query-row tiles per core (4)
    n_jjt = sh // P           # kv-row tiles per core (4)
    n_eh = dim // 512         # 512-wide output column halves (2)
    EH = 512 if dim >= 512 else dim
    n_eh = max(1, dim // EH)
    n_jt = nq // P            # total kv j tiles (32)
    JG = 4                    # j-tiles per PV V-chunk
    n_jg = n_jt // JG         # V chunk count (8)
    IT_GROUP = 2              # i-tiles per PV psum group
    scale = 1.0 / float(np.sqrt(dim))

    nc = bacc.Bacc(num_devices=cores)

    # --- external I/O (per core: row shards of q/k/v, full weights) ---
    q_ext = nc.declare_dram_parameter("q", [sh, dim], f32, isOutput=False)
    k_ext = nc.declare_dram_parameter("k", [sh, dim], f32, isOutput=False)
    v_ext = nc.declare_dram_parameter("v", [sh, dim], f32, isOutput=False)
    wq_ext = nc.declare_dram_parameter("W_q", [dim, dim], f32, isOutput=False)
    wk_ext = nc.declare_dram_parameter("W_k", [dim, dim], f32, isOutput=False)
    wv_ext = nc.declare_dram_parameter("W_v", [dim, dim], f32, isOutput=False)
    out_ext = nc.declare_dram_parameter("out", [sh, dim], f32, isOutput=True)

    # --- internal DRAM for collectives ---
    # bounce/gather layouts are partition-major so every DMA touches
    # 4-8 KB contiguous runs per partition (few, large descriptors).
    # Exactly TWO collectives: ticks of first and last collectives post
    # promptly; middle ticks were observed posting ~30us late.
    n_dtt = dim // P   # d-tiles (8)
    n_vjt = sh // P    # j-tiles per core (4)
    bounce_k = nc.dram_tensor("bounce_k", [P * n_dtt, sh], bf16)
    bounce_v = nc.dram_tensor("bounce_v", [P * n_vjt, dim], bf16)
    gath_k = nc.dram_tensor("gath_k", [cores * P * n_dtt, sh], bf16, addr_space="Shared")
    gath_v = nc.dram_tensor("gath_v", [cores * P * n_vjt, dim], bf16, addr_space="Shared")

    rg = [list(range(cores))]

    with TileContext(nc) as tc:
        with (
            tc.tile_pool(name="const", bufs=1) as constp,
            tc.tile_pool(name="qt", bufs=1) as qtp,
            tc.tile_pool(name="stats", bufs=1) as statp,
        ):
            ident_f = constp.tile([P, P], f32, tag="idf", name="idf")
            make_identity(nc, ident_f)
            ident_b = constp.tile([P, P], bf16, tag="idb", name="idb")
            make_identity(nc, ident_b)

            qthi = qtp.tile([P, n_dt, sh], bf16, tag="qthi", name="qthi")

            def load_w_f32r(w_ext, wpool, wstage):
                """Load a [dim, dim] f32 weight into SBUF as float32r laid
                out [c_in=128, ct, d] (scalar copy performs the rounding)."""
                wr = wpool.tile([P, n_ct, dim], f32r, tag="wr", name="wr")
                wsrc = w_ext.rearrange("(ct p) d -> p ct d", p=P)
                for ct in range(n_ct):
                    stg = wstage.tile([P, dim], f32, tag="wstg", name="wstg")
                    nc.sync.dma_start(stg[:], wsrc[:, ct])
                    nc.scalar.copy(wr[:, ct], stg[:])
                return wr

            def load_transpose_f32r(x_ext, tpool, iost, tpsum):
                """Load a [sh, dim] f32 input, transpose on PE, round to
                float32r in SBUF laid out [c_in=128, ct, row]."""
                xt = tpool.tile([P, n_ct, sh], f32r, tag="xt", name="xt")
                xsrc = x_ext.rearrange("(it p) c -> p it c", p=P)
                for it in range(sh // P):
                    stg = iost.tile([P, dim], f32, tag="iostg", name="iostg")
                    nc.sync.dma_start(stg[:], xsrc[:, it])
                    for ct in range(n_ct):
                        ps = tpsum.tile([P, P], f32, tag="tps", name="tps")
                        nc.tensor.transpose(ps[:], stg[:, ct * P:(ct + 1) * P], ident_f)
                        nc.vector.tensor_copy(xt[:, ct, it * P:(it + 1) * P], ps[:])
                return xt

            with (
                tc.tile_pool(name="wstage", bufs=2) as wstage,
                tc.tile_pool(name="w", bufs=2) as wpool,
                tc.tile_pool(name="iost", bufs=4) as iost,
                tc.tile_pool(name="tin", bufs=2) as tpool,
                tc.tile_pool(name="kvout", bufs=2) as kvout,
                tc.tile_pool(name="tpsum", bufs=2, space="PSUM") as tpsum,
                tc.tile_pool(name="ppsum", bufs=1, space="PSUM") as ppsum,
                tc.tile_pool(name="kproj", bufs=4, space="PSUM") as kprojp,
            ):
                # ---- K path: project K^T shard (f32r matmul, fp16
                # correctly-rounded result), bounce out, AG ----
                # k transposes interleave with the projection ct-by-ct (two
                # dtt-half passes, 4 open PSUM groups each) so the PE never
                # serializes transpose-then-project and the K all-gather
                # triggers as early as possible
                ksrc4 = k_ext.rearrange("(it p) c -> p it c", p=P)
                kstg = []
                for it4 in range(sh // P):
                    stg = iost.tile([P, dim], f32, tag="iostg", name="iostg")
                    nc.sync.dma_start(stg[:], ksrc4[:, it4])
                    kstg.append(stg)
                wkr = load_w_f32r(wk_ext, wpool, wstage)
                ktr = tpool.tile([P, n_ct, sh], f32r, tag="xt", name="ktr")

                kt_hi_loc = kvout.tile([P, n_dt, sh], bf16, tag="kthi_loc", name="kthi_loc")
                for half in range(2):
                    dtts = range(half * (n_dt // 2), (half + 1) * (n_dt // 2))
                    pss = {
                        dtt: kprojp.tile([P, sh], f32, tag="kpps", name="kpps")
                        for dtt in dtts
                    }
                    for ct in range(n_ct):
                        if half == 0:
                            for it4 in range(sh // P):
                                tps = tpsum.tile([P, P], f32, tag="tps", name="tps")
                                nc.tensor.transpose(
                                    tps[:], kstg[it4][:, ct * P:(ct + 1) * P],
                                    ident_f,
                                )
                                nc.vector.tensor_copy(
                                    ktr[:, ct, it4 * P:(it4 + 1) * P], tps[:]
                                )
                        for dtt in dtts:
                            dsl = slice(dtt * P, (dtt + 1) * P)
                            nc.tensor.matmul(
                                pss[dtt][:], wkr[:, ct, dsl], ktr[:, ct],
                                start=(ct == 0), stop=(ct == n_ct - 1),
                            )
                    for dtt in dtts:
                        nc.scalar.copy(kt_hi_loc[:, dtt], pss[dtt][:])

                bk = bounce_k.rearrange("(p dtt) jj -> p dtt jj", p=P)
                nc.sync.dma_start(bk[:], kt_hi_loc[:])
                nc.gpsimd.collective_compute(
                    "AllGather", mybir.AluOpType.bypass, replica_groups=rg,
                    ins=[bounce_k.ap().opt()], outs=[gath_k.ap().opt()],
                )

                # ---- V path: project V shard (f32r), bounce, AG right
                # behind the K gathers so it lands before PV needs it ----
                wvr = load_w_f32r(wv_ext, wpool, wstage)
                vtr = load_transpose_f32r(v_ext, tpool, iost, tpsum)

                v_loc = kvout.tile([P, n_jjt, dim], bf16, tag="v_loc", name="v_loc")
                for jjt in range(n_jjt):
                    jsl = slice(jjt * P, (jjt + 1) * P)
                    for eh in range(n_eh):
                        ps = ppsum.tile([P, EH], f32, tag="ppsv", name="ppsv")
                        esl = slice(eh * EH, (eh + 1) * EH)
                        for ct in range(n_ct):
                            nc.tensor.matmul(
                                ps[:], vtr[:, ct, jsl], wvr[:, ct, esl],
                                start=(ct == 0), stop=(ct == n_ct - 1),
                            )
                        nc.scalar.copy(v_loc[:, jjt, esl], ps[:])

                bv = bounce_v.rearrange("(p jjt) e -> p jjt e", p=P)
                nc.sync.dma_start(bv[:], v_loc[:])
                # (the V all-gather itself is issued after the score chunk
                # loads: any DRAM read issued after a collective waits for
                # its tick, so issuing AG_v here would stall the khi loads)

                # ---- Q path (local only): project Q^T (f32r) ----
                wqr = load_w_f32r(wq_ext, wpool, wstage)
                qtr = load_transpose_f32r(q_ext, tpool, iost, tpsum)

                for dtt in range(n_dt):
                    ps = ppsum.tile([P, sh], f32, tag="pps", name="pps")
                    dsl = slice(dtt * P, (dtt + 1) * P)
                    for ct in range(n_ct):
                        nc.tensor.matmul(
                            ps[:], wqr[:, ct, dsl], qtr[:, ct],
                            start=(ct == 0), stop=(ct == n_ct - 1),
                        )
                    nc.scalar.copy(qthi[:, dtt], ps[:])

            # ================= attention phase =================
            m_t = [
                [
                    statp.tile([P, 1], f32, tag=f"m{it}_{pp}", name=f"m{it}_{pp}")
                    for pp in range(2)
                ]
                for it in range(n_it)
            ]
            bias_t = [statp.tile([P, 1], f32, tag=f"b{it}", name=f"b{it}") for it in range(n_it)]
            ell_t = [statp.tile([P, 1], f32, tag=f"l{it}", name=f"l{it}") for it in range(n_it)]
            rl_t = [statp.tile([P, 1], f32, tag=f"r{it}", name=f"r{it}") for it in range(n_it)]

            gk = gath_k.rearrange("(r p dtt) jj -> r p dtt jj", r=cores, p=P)
            gv = gath_v.rearrange("(r p jj) e -> r p jj e", r=cores, p=P)

            with (
                tc.tile_pool(name="prow", bufs=n_it) as prow,
                tc.tile_pool(name="ptp", bufs=1) as ptp,
                tc.tile_pool(name="vchunk", bufs=3) as vchunk,
                tc.tile_pool(name="opool", bufs=2) as opool,
            ):
                _schunk_cm = tc.tile_pool(name="schunk", bufs=1)
                schunk = _schunk_cm.__enter__()
                _srow_cm = tc.tile_pool(name="srow", bufs=2)
                srow = _srow_cm.__enter__()

                # ---- scores: the whole gathered K^T comes in as just TWO
                # big DMAs (fewest instructions => fewest chances for the
                # scheduler's completion-count thresholds to couple them to
                # the V collective); V all-gather issued after them ----
                _spsum_cm = tc.tile_pool(name="spsum", bufs=6, space="PSUM")
                spsum = _spsum_cm.__enter__()
                _ptpsum_cm = tc.tile_pool(name="ptpsum", bufs=2, space="PSUM")
                ptpsum = _ptpsum_cm.__enter__()
                gkb = gath_k.rearrange("(r p dtt) jj -> p r dtt jj", r=cores, p=P)
                # four quarter loads alternating HWDGE queues: the first
                # 2 MB lands ~10us after the K gather, and quarters arrive
                # in exactly the order the it-outer score loop consumes them
                qc = cores // 4
                khi_q = []
                for qi in range(4):
                    kq = schunk.tile(
                        [P, qc, n_dt, sh], bf16, tag=f"kq{qi}", name=f"kq{qi}"
                    )
                    eng = nc.sync if qi % 2 == 0 else nc.scalar
                    eng.dma_start(kq[:], gkb[:, qi * qc:(qi + 1) * qc])
                    khi_q.append(kq)
                nc.gpsimd.collective_compute(
                    "AllGather", mybir.AluOpType.bypass, replica_groups=rg,
                    ins=[bounce_v.ap().opt()], outs=[gath_v.ap().opt()],
                )

                # it-outer: S rows complete one i-tile at a time, so exp and
                # the P^T transposes for tile i overlap the scores of i+1
                p_sb = []
                pts = {}

                def emit_transposes(it):
                    pt = ptp.tile([P, n_jt, P], bf16, tag=f"pt{it}", name=f"pt{it}")
                    for jt in range(n_jt):
                        tps = ptpsum.tile([P, P], bf16, tag="ptps", name="ptps")
                        nc.tensor.transpose(
                            tps[:], p_sb[it][:, jt * P:(jt + 1) * P], ident_b
                        )
                        nc.vector.tensor_copy(pt[:, jt], tps[:])
                    pts[it] = pt

                for it in range(n_it):
                    isl = slice(it * P, (it + 1) * P)
                    s_it = srow.tile([P, nq], f32, tag="s", name="s")
                    for rr in range(cores):
                        ksrc = khi_q[rr // qc]
                        kidx = rr % qc
                        ps = spsum.tile([P, sh], f32, tag="sps", name="sps")
                        for dtt in range(n_dt):
                            nc.tensor.matmul(
                                ps[:], qthi[:, dtt, isl], ksrc[:, kidx, dtt],
                                start=(dtt == 0), stop=(dtt == n_dt - 1),
                            )
                        if rr == 0:
                            nc.vector.reduce_max(
                                m_t[it][0][:], ps[:], axis=mybir.AxisListType.X
                            )
                        else:
                            nc.vector.reduce_max(
                                m_t[it][1][:], ps[:], axis=mybir.AxisListType.X
                            )
                            nc.vector.tensor_max(
                                m_t[it][0][:], m_t[it][0][:], m_t[it][1][:]
                            )
                        nc.scalar.copy(
                            s_it[:, rr * sh:(rr + 1) * sh], ps[:]
                        )
                    p_it = prow.tile([P, nq], bf16, tag="p", name="p")
                    p_sb.append(p_it)
                    nc.vector.tensor_scalar_mul(
                        bias_t[it][:], m_t[it][0][:], -scale
                    )
                    nc.scalar.activation(
                        p_it[:], s_it[:],
                        mybir.ActivationFunctionType.Exp,
                        bias=bias_t[it][:], scale=scale,
                        accum_out=ell_t[it][:],
                    )
                    nc.vector.reciprocal(rl_t[it][:], ell_t[it][:])
                    if it >= 1:
                        emit_transposes(it - 1)
                emit_transposes(n_it - 1)

                _ptpsum_cm.__exit__(None, None, None)
                _spsum_cm.__exit__(None, None, None)
                _srow_cm.__exit__(None, None, None)
                _schunk_cm.__exit__(None, None, None)
                # ---- O = (P @ V) / ell, single V pass, 8 psum banks ----

                _pvpsum_cm = tc.tile_pool(name="pvpsum", bufs=n_it * n_eh, space="PSUM")
                pvpsum = _pvpsum_cm.__enter__()
                pso = {
                    (it, eh): pvpsum.tile([P, EH], f32, tag="pvps", name="pvps")
                    for it in range(n_it) for eh in range(n_eh)
                }
                for r in range(cores):
                    vc = vchunk.tile([P, n_jjt, dim], bf16, tag="vc", name="vc")
                    # V chunks alternate between the two HWDGE queues so the
                    # 8 MB stream runs at double single-queue bandwidth
                    eng = nc.scalar if r % 2 == 0 else nc.sync
                    eng.dma_start(vc[:], gv[r])
                    for jj in range(n_jjt):
                        jt = r * n_jjt + jj
                        for it in range(n_it):
                            for eh in range(n_eh):
                                esl = slice(eh * EH, (eh + 1) * EH)
                                nc.tensor.matmul(
                                    pso[(it, eh)][:],
                                    pts[it][:, jt],
                                    vc[:, jj, esl],
                                    start=(r == 0 and jj == 0),
                                    stop=(r == cores - 1 and jj == n_jjt - 1),
                                )
                for it in range(n_it):
                    o_sb = opool.tile([P, dim], f32, tag="o", name="o")
                    for eh in range(n_eh):
                        esl = slice(eh * EH, (eh + 1) * EH)
                        nc.vector.tensor_scalar_mul(
                            o_sb[:, esl], pso[(it, eh)][:], rl_t[it][:]
                        )
                    nc.sync.dma_start(out_ext[it * P:(it + 1) * P, :], o_sb[:])
                _pvpsum_cm.__exit__(None, None, None)

    return nc


_CACHE = {}
RUN_KW = {}


def _get_nc():
    if "nc" not in _CACHE:
        _CACHE["nc"] = build_attention()
    return _CACHE["nc"]


def kernel(**inputs):
    from concourse.bass_utils import run_bass_kernel_spmd

    q = np.ascontiguousarray(np.asarray(inputs["q"], dtype=np.float32))
    k = np.ascontiguousarray(np.asarray(inputs["k"], dtype=np.float32))
    v = np.ascontiguousarray(np.asarray(inputs["v"], dtype=np.float32))
    W_q = np.ascontiguousarray(np.asarray(inputs["W_q"], dtype=np.float32))
    W_k = np.ascontiguousarray(np.asarray(inputs["W_k"], dtype=np.float32))
    W_v = np.ascontiguousarray(np.asarray(inputs["W_v"], dtype=np.float32))

    sh = N_Q // CORES
    in_maps = []
    for r in range(CORES):
        sl = slice(r * sh, (r + 1) * sh)
        in_maps.append({
            "q": q[sl], "k": k[sl], "v": v[sl],
            "W_q": W_q, "W_k": W_k, "W_v": W_v,
        })

    nc = _get_nc()
    if not nc.is_finalized():
        nc.finalize()
    res = run_bass_kernel_spmd(nc, in_maps, core_ids=list(range(CORES)), **RUN_KW)
    _CACHE["last_result"] = res
    out = np.concatenate([res.results[r]["out"] for r in range(CORES)], axis=0)
    return out


if __name__ == "__main__":
    import reference

    inputs = {kk: np.asarray(vv) for kk, vv in reference.setup_inputs().items()}
    out = kernel(**inputs)
    print("out shape:", out.shape, out.dtype)

